# revision 1
# baseline (speedup 1.0000x reference)
"""Trainium2 Bass kernel for nn_EnhancedObj (gnn_message_passing).

Per batch sample (data-parallel over 8 cores, one sample per core):
    ve  = LN(tanh(visual @ W_v + b_v))                  [64, 2048]
    oe  = LN(tanh(obj_flat @ W_o + b_o))                [2304, 2048]
    adj = softmax_n(oe @ ve^T / sqrt(2048))             [2304, 64]
    out = LN(tanh(adj^T @ oe + ve))                     [64, 2048]

All matmuls run in fp16 (fp32 PSUM accumulate) — fp16 streams at the
same 1 col/cycle as bf16 on the TRN2 PE but carries a 10-bit mantissa
(verified vs fp32 reference: maxabs ~3e-3 on a ~1.4-absmax output,
rel-fro ~3.6e-4).  Softmax and all LayerNorm statistics are fp32.

Schedule: ONE fused PE stream.  Phase B starts immediately (chunk 0
paced by W_o slice arrival); the visual branch (A) is emitted between
object chunks 3 and 4, consuming W_v that streamed in behind W_o; the
adjacency (C) and aggregation (D) matmuls interleave into the stream
two chunks at a time, with oe transposes riding the sync HWDGE queue
behind the weight streams.  Softmax uses unnormalized exp weights (logits are O(1)-
bounded, so no max subtraction); the aggregation is rescaled by the
global 1/sum at the end, so nothing serializes behind a full softmax.
PSUM: 3 banks B quarters + 1 bank C + 4 banks (A, then D agg) = 8.

The device kernel assumes the spec's deterministic fills (zero biases,
unit gains).  If non-trivial bias/gain vectors are ever passed, we
fall back to an exact fp32 numpy implementation.
"""

import numpy as np

F16 = np.float16

BS = 8          # batch (== number of cores)
F = 64          # win_len (frames)
OBJ = 36        # objects per frame
D = 2048        # feature dim
N = F * OBJ     # 2304 objects per sample
NCH = N // 128  # 18 object-row chunks
NW = NCH // 2   # 9 two-chunk adjacency windows
KC = D // 128   # 16 contraction chunks
DW = 512        # matmul moving width (one PSUM bank of fp32)
ND = D // DW    # 4 output-column groups
LN_EPS = 1e-5

_BUILD_CACHE = {}


def _f32(x):
    return np.ascontiguousarray(np.asarray(x), dtype=np.float32)


def _klc_layout(w):
    """[D, M] -> [128(kl), KC*M] with element (kl, kc, m) = w[kc*128+kl, m]."""
    d, m = w.shape
    assert d == D
    return w.reshape(KC, 128, m).transpose(1, 0, 2).reshape(128, KC * m)


def _build():
    """Build + compile the SPMD Bass program (trivial-fill fast path)."""
    if "nc" in _BUILD_CACHE:
        return _BUILD_CACHE["nc"]

    import concourse.bacc as bacc
    import concourse.tile as tile
    from concourse import mybir

    f32 = mybir.dt.float32
    f16 = mybir.dt.float16
    AF = mybir.ActivationFunctionType
    AX = mybir.AxisListType
    OP = mybir.AluOpType

    nc = bacc.Bacc("TRN2", target_bir_lowering=False, debug=False, num_devices=BS)

    objT_d = nc.dram_tensor("objT", [NCH, 128, KC * 128], f16, kind="ExternalInput").ap()
    wo_d = nc.dram_tensor("Wo", [128, KC * D], f16, kind="ExternalInput").ap()
    wv_d = nc.dram_tensor("Wv", [128, KC * D], f16, kind="ExternalInput").ap()
    vt_d = nc.dram_tensor("vT", [128, KC * F], f16, kind="ExternalInput").ap()
    out_d = nc.dram_tensor("out", [F, D], f32, kind="ExternalOutput").ap()

    inv_sqrt_d = 1.0 / float(np.sqrt(D))

    # adjacency (C) / aggregation (D) emission points: window w covers
    # object chunks (2w, 2w+1); C(w) needs both transposed + veT (ready
    # after chunk 4); D(w) follows C(w) one chunk later.
    sched = {}
    for w in range(NW):
        c_at = max(2 * w + 3, 5 + (0 if w < 3 else 0)) if w >= 3 else 5 + w
        c_at = min(c_at, NCH - 1) if w < NW - 1 else NCH  # NCH == post-loop
        d_at = c_at + 1
        if c_at < NCH:
            sched.setdefault(c_at, []).append(("C", w))
        if d_at < NCH:
            sched.setdefault(d_at, []).append(("D", w))

    with tile.TileContext(nc) as tc:
        with tc.tile_pool(name="persist", bufs=1) as persist, \
             tc.tile_pool(name="stats", bufs=2) as stats_pool:

            eps128 = persist.tile([128, 1], f32)
            nc.vector.memset(eps128, LN_EPS)

            def layer_norm_to(t_in, rows, out_tile):
                """LN over the free dim of t_in[:rows] -> out_tile (casts)."""
                st = stats_pool.tile([128, ND, nc.vector.BN_STATS_DIM], f32, tag="st")
                for j in range(ND):
                    nc.vector.bn_stats(out=st[:rows, j, :],
                                       in_=t_in[:rows, j * DW:(j + 1) * DW])
                mvr = stats_pool.tile([128, 3], f32, tag="mvr")
                nc.vector.bn_aggr(out=mvr[:rows, 0:2], in_=st[:rows])
                nc.scalar.activation(out=mvr[:rows, 2:3], in_=mvr[:rows, 1:2],
                                     func=AF.Sqrt, bias=eps128[:rows], scale=1.0)
                nc.vector.reciprocal(out=mvr[:rows, 2:3], in_=mvr[:rows, 2:3])
                nc.vector.tensor_scalar(
                    out=out_tile[:rows], in0=t_in[:rows],
                    scalar1=mvr[:rows, 0:1], scalar2=mvr[:rows, 2:3],
                    op0=OP.subtract, op1=OP.mult)

            ve_nat = persist.tile([F, D], f32)          # LN'd visual embedding
            veT = persist.tile([128, KC, F], f16)       # transposed, for adjacency
            oe_nat = persist.tile([128, NCH, D], f16)   # LN'd object embeddings
            psum_w = persist.tile([F, NW + 1], f32)     # per-window exp sums

            with tc.tile_pool(name="wo", bufs=1) as wop, \
                 tc.tile_pool(name="objs", bufs=2) as objp, \
                 tc.tile_pool(name="psB", bufs=3, space="PSUM") as psB, \
                 tc.tile_pool(name="psC", bufs=1, space="PSUM") as psC, \
                 tc.tile_pool(name="ew", bufs=1) as ewp, \
                 tc.tile_pool(name="tmpB", bufs=2) as tmpB:
                wo = wop.tile([128, KC * D], f16)

                # DMA plan: objT loads ride the scalar HWDGE queue; W_o,
                # then W_v, then all transposes stream on the sync queue
                # (W_v's slot-waits resolve before any transpose is due).
                obj_tiles = {}

                def load_objT(nch):
                    t = objp.tile([128, KC, 128], f16, name="objT", tag="objT")
                    nc.scalar.dma_start(out=t, in_=objT_d[nch])
                    obj_tiles[nch] = t

                load_objT(0)
                load_objT(1)
                for kc in range(KC):
                    nc.sync.dma_start(out=wo[:, kc * D:(kc + 1) * D],
                                      in_=wo_d[:, kc * D:(kc + 1) * D])

                win_tiles = {}
                en_tiles = {}
                pending_transpose = []

                def emit_transpose(nch):
                    w = nch // 2
                    if w not in win_tiles:
                        win_tiles[w] = tc_win.tile([128, 2, KC, 128], f16,
                                                   name="winT", tag="winT")
                    nc.sync.dma_start(out=win_tiles[w][:, nch % 2, :, :],
                                      in_=oe_nat[:, nch, :], transpose=True)

                def emit_chunk_B(nch):
                    objT_nc = obj_tiles.pop(nch)
                    if nch + 2 < NCH:
                        load_objT(nch + 2)
                    tB = tmpB.tile([128, D], f16, tag="tB")
                    # quarter-width PSUM tiles (1 bank each, 3 bufs) so each
                    # quarter's tanh overlaps the next quarter's matmuls.
                    for q in range(ND):
                        pq = psB.tile([128, DW], f32, tag="psb")
                        for kc in range(KC):
                            nc.tensor.matmul(
                                pq,
                                lhsT=objT_nc[:, kc, :],
                                rhs=wo[:, kc * D + q * DW: kc * D + (q + 1) * DW],
                                start=(kc == 0), stop=(kc == KC - 1))
                        nc.scalar.activation(out=tB[:, q * DW:(q + 1) * DW],
                                             in_=pq, func=AF.Tanh)
                    layer_norm_to(tB, 128, oe_nat[:, nch, :])

                def emit_window_C(w):
                    """Adjacency + exp for window w (chunks 2w, 2w+1)."""
                    wt = win_tiles.pop(w)
                    padj = psC.tile([F, 256], f32, tag="padj")
                    for kc in range(KC):
                        nc.tensor.matmul(
                            padj,
                            lhsT=veT[:, kc, :],
                            rhs=wt[:, :, kc, :],
                            start=(kc == 0), stop=(kc == KC - 1))
                    # Unnormalized softmax weights: logits are O(1)-bounded
                    # so exp without max-subtraction is safe; accum_out
                    # collects this window's exp-sum for free.
                    ewt = ewp.tile([F, 256], f32, tag="ew")
                    nc.scalar.activation(out=ewt, in_=padj, func=AF.Exp,
                                         scale=inv_sqrt_d,
                                         accum_out=psum_w[:, w:w + 1])
                    e16 = ewp.tile([F, 256], f16, tag="e16")
                    nc.vector.tensor_copy(out=e16, in_=ewt)
                    en = ewp.tile([128, 2, F], f16, tag="en", bufs=2)
                    # [64, 256] -> rows n: [nw, j, f]
                    nc.sync.dma_start(out=en, in_=e16, transpose=True)
                    en_tiles[w] = en

                def emit_window_D(w):
                    """Aggregation matmuls for window w into ps_agg."""
                    en = en_tiles.pop(w)
                    for j in range(2):
                        for dd in range(ND):
                            nc.tensor.matmul(
                                ps_agg[:, dd * DW:(dd + 1) * DW],
                                lhsT=en[:, j, :],
                                rhs=oe_nat[:, 2 * w + j, dd * DW:(dd + 1) * DW],
                                start=(w == 0 and j == 0),
                                stop=(w == NW - 1 and j == 1))

                # ---- object chunks 0-3 (W_o-arrival paced) ------------
                with tc.tile_pool(name="wv", bufs=4) as wvp, \
                     tc.tile_pool(name="vt", bufs=1) as vtp, \
                     tc.tile_pool(name="psA", bufs=1, space="PSUM") as psA, \
                     tc.tile_pool(name="tmpA", bufs=1) as tmpA:
                    vt = vtp.tile([128, KC, F], f16)
                    nc.scalar.dma_start(out=vt, in_=vt_d)

                    # W_v streams behind W_o on the sync queue; phase A's
                    # matmuls (emitted below) consume it at chunk-4 time.
                    wv_slices = []
                    for kc in range(KC):
                        wv_k = wvp.tile([128, D], f16, tag="wvk")
                        nc.sync.dma_start(out=wv_k, in_=wv_d[:, kc * D:(kc + 1) * D])
                        wv_slices.append(wv_k)

                    for nch in range(4):
                        emit_chunk_B(nch)
                        pending_transpose.append(nch)

                    # ---- phase A: visual branch -----------------------
                    ps_ve = psA.tile([F, D], f32)
                    for kc in range(KC):
                        for dd in range(ND):
                            nc.tensor.matmul(
                                ps_ve[:, dd * DW:(dd + 1) * DW],
                                lhsT=vt[:, kc, :],
                                rhs=wv_slices[kc][:, dd * DW:(dd + 1) * DW],
                                start=(kc == 0), stop=(kc == KC - 1))
                    tA = tmpA.tile([F, D], f32)
                    nc.scalar.activation(out=tA, in_=ps_ve, func=AF.Tanh)
                    layer_norm_to(tA, F, ve_nat)
                    ve_bf = tmpB.tile([F, D], f16, tag="tB")
                    nc.vector.tensor_copy(out=ve_bf, in_=ve_nat)
                    # [64, 2048] -> rows d=(kc*128+kl): [kl, kc, f]
                    nc.sync.dma_start(out=veT, in_=ve_bf, transpose=True)

                # ---- object chunks 4-17 with fused C/D ----------------
                with tc.tile_pool(name="win", bufs=3) as tc_win, \
                     tc.tile_pool(name="psD", bufs=1, space="PSUM") as psD:
                    ps_agg = psD.tile([F, D], f32)

                    for nch in range(4, NCH):
                        emit_chunk_B(nch)
                        # drain deferred chunk 0-3 transposes two at a time
                        # behind the current chunk's matmuls
                        for _ in range(min(2, len(pending_transpose))):
                            emit_transpose(pending_transpose.pop(0))
                        emit_transpose(nch)
                        for kind, w in sched.get(nch, []):
                            (emit_window_C if kind == "C" else emit_window_D)(w)
                    # Drain the last two windows: the final window's
                    # adjacency runs chunk-16's half while chunk-17's
                    # transpose is in flight, with D(7) filling the gap.
                    wt = win_tiles.pop(NW - 1)
                    padj = psC.tile([F, 256], f32, tag="padj")
                    for kc in range(KC):
                        nc.tensor.matmul(
                            padj[:, 0:128], lhsT=veT[:, kc, :],
                            rhs=wt[:, 0:1, kc, :],
                            start=(kc == 0), stop=(kc == KC - 1))
                    emit_window_D(NW - 2)
                    for kc in range(KC):
                        nc.tensor.matmul(
                            padj[:, 128:256], lhsT=veT[:, kc, :],
                            rhs=wt[:, 1:2, kc, :],
                            start=(kc == 0), stop=(kc == KC - 1))
                    ewt = ewp.tile([F, 256], f32, tag="ew")
                    nc.scalar.activation(out=ewt, in_=padj, func=AF.Exp,
                                         scale=inv_sqrt_d,
                                         accum_out=psum_w[:, NW - 1:NW])
                    e16 = ewp.tile([F, 256], f16, tag="e16")
                    nc.vector.tensor_copy(out=e16, in_=ewt)
                    en = ewp.tile([128, 2, F], f16, tag="en", bufs=2)
                    nc.sync.dma_start(out=en, in_=e16, transpose=True)
                    en_tiles[NW - 1] = en
                    emit_window_D(NW - 1)

                    # ---- finalize: rescale by 1/sum, residual, LN -----
                    nc.vector.reduce_sum(out=psum_w[:, NW:NW + 1],
                                         in_=psum_w[:, :NW], axis=AX.X)
                    nc.vector.reciprocal(out=psum_w[:, NW:NW + 1],
                                         in_=psum_w[:, NW:NW + 1])
                    tD = tc_win.tile([F, D], f32, tag="winT")
                    nc.vector.scalar_tensor_tensor(
                        out=tD, in0=ps_agg, scalar=psum_w[:, NW:NW + 1],
                        in1=ve_nat, op0=OP.mult, op1=OP.add)
                    nc.scalar.activation(out=tD, in_=tD, func=AF.Tanh)
                    out_f = tc_win.tile([F, D], f32, tag="winT")
                    layer_norm_to(tD, F, out_f)
                    nc.sync.dma_start(out=out_d, in_=out_f)

    nc.compile()
    _BUILD_CACHE["nc"] = nc
    return nc


def _numpy_fallback(inputs):
    """Exact fp32 implementation for non-trivial bias/gain fills."""
    def ln(x, g, b, eps=LN_EPS):
        mu = x.mean(-1, keepdims=True)
        var = x.var(-1, keepdims=True)
        return (x - mu) / np.sqrt(var + eps) * g + b

    vf = _f32(inputs["visual_feats"])
    of = _f32(inputs["obj_feats"])
    W_v, b_v = _f32(inputs["W_v"]), _f32(inputs["b_v"])
    W_o, b_o = _f32(inputs["W_o"]), _f32(inputs["b_o"])
    out = np.zeros((BS, F, D), np.float32)
    for i in range(BS):
        ve = ln(np.tanh(vf[i] @ W_v + b_v), _f32(inputs["ln_v_g"]), _f32(inputs["ln_v_b"]))
        oe = ln(np.tanh(of[i].reshape(N, D) @ W_o + b_o),
                _f32(inputs["ln_o_g"]), _f32(inputs["ln_o_b"]))
        adj = oe @ ve.T / np.sqrt(D)
        adj = np.exp(adj - adj.max(0, keepdims=True))
        adj /= adj.sum(0, keepdims=True)
        out[i] = ln(np.tanh(adj.T @ oe + ve),
                    _f32(inputs["ln_ov_g"]), _f32(inputs["ln_ov_b"]))
    return out


def _prep_core_inputs(visual, obj_flat, shared):
    """Host-side per-sample layout prep. visual [64,2048] f32, obj_flat [2304,2048] f32."""
    m = {
        "objT": np.ascontiguousarray(
            obj_flat.reshape(NCH, 128, KC, 128).transpose(0, 3, 2, 1)
        ).astype(F16).reshape(NCH, 128, KC * 128),
        "vT": np.ascontiguousarray(
            _klc_layout(np.ascontiguousarray(visual.T))).astype(F16),
    }
    m.update(shared)
    return m


def run_kernel(inputs, trace=False):
    """Returns (out [8, 64, 2048] fp32, exec_time_ns or None)."""
    from concourse import bass_utils

    vecs = {k: _f32(inputs[k]) for k in
            ["b_v", "b_o", "ln_v_b", "ln_o_b", "ln_ov_b"]}
    gains = {k: _f32(inputs[k]) for k in ["ln_v_g", "ln_o_g", "ln_ov_g"]}
    trivial = (all(np.all(v == 0) for v in vecs.values())
               and all(np.all(g == 1) for g in gains.values()))
    if not trivial:
        return _numpy_fallback(inputs), None

    visual = _f32(inputs["visual_feats"])            # [8, 64, 2048]
    obj = _f32(inputs["obj_feats"])                  # [8, 64, 36, 2048]
    W_v = _f32(inputs["W_v"])
    W_o = _f32(inputs["W_o"])

    nc = _build()

    shared = {
        "Wo": np.ascontiguousarray(_klc_layout(W_o)).astype(F16),
        "Wv": np.ascontiguousarray(_klc_layout(W_v)).astype(F16),
    }
    in_maps = [
        _prep_core_inputs(visual[c], obj[c].reshape(N, D), shared)
        for c in range(BS)
    ]

    res = bass_utils.run_bass_kernel_spmd(
        nc, in_maps, core_ids=list(range(BS)), trace=trace)
    out = np.stack([res.results[c]["out"] for c in range(BS)], axis=0)
    return out.astype(np.float32), res.exec_time_ns


def kernel(**inputs):
    out, _ = run_kernel(inputs, trace=False)
    return out



# revision 10
# speedup vs baseline: 1.6290x; 1.6290x over previous
"""Trainium2 Bass kernel for nn_EnhancedObj (gnn_message_passing).

Per batch sample (data-parallel over 8 cores, one sample per core):
    ve  = LN(tanh(visual @ W_v + b_v))                  [64, 2048]
    oe  = LN(tanh(obj_flat @ W_o + b_o))                [2304, 2048]
    adj = softmax_n(oe @ ve^T / sqrt(2048))             [2304, 64]
    out = LN(tanh(adj^T @ oe + ve))                     [64, 2048]

The dominant matmuls run in fp8-e4m3 with DoubleRow perf mode (two
128-deep k-tiles per instruction, 2x the fp16 PE rate):
  B (obj @ W_o):  obj scaled x16, W_o scaled x64 on the host; the tanh
                  activation folds the 1/1024 back out.  PSUM is fp32.
  C (adjacency):  computed TRANSPOSED, adjT[n,f] = sum_d oeT * veT with
                  both operands fp8(16*x); full 128-partition output and
                  the layout D wants for its stationary operand.
  D (aggregate):  lhsT = fp8 exp-weights, rhs = fp8(16*oe); scales cancel.
The visual branch (A) stays fp16 (its error would flow straight into the
residual).  Verified against the fp32 reference: rel-fro err ~3.0e-3.

Softmax uses unnormalized exp weights stored as fp8(w/16) (the -ln16
bias keeps peak logits ~e^5.5 inside e4m3 range); per-frame sums come
free from a ones-column DoubleRow matmul accumulated in PSUM, and the
final rescale divides by 16*sum.  Exp runs in 3 batches (chunks 0-7,
8-15, 16-17) so the scalar engine only swaps activation tables 6 times
instead of 40 (each ACT_TABLE_LOAD is ~1.5us).

LayerNorm statistics: bn_stats/bn_aggr on DVE; 1/sqrt(var+eps) via the
int32 bit-trick plus two Newton iterations, entirely on DVE (keeps the
scalar engine's table pinned to Tanh).

Schedule: one fused PE stream.  Per chunk nch: 32 DoubleRow B matmuls
-> tanh quarters -> LN -> (DVE) casts to fp8 natural + (DMA, vector
queue) transpose -> fp8 cast.  C(nch) rides one chunk behind; the A
branch is emitted between chunks 3 and 4 (paced by the W_v stream on the
sync queue); D windows interleave after each exp batch.
PSUM banks: 2 (B quarters) + 1 (C slots) + 4 (agg) + 1 (sums) = 8,
with the A-phase 4-bank accumulator closed before agg/sums open.

The device kernel assumes the spec's deterministic fills (zero biases,
unit gains).  If non-trivial bias/gain vectors are ever passed, we
fall back to an exact fp32 numpy implementation.
"""

import numpy as np
import ml_dtypes

F16 = np.float16
F8 = ml_dtypes.float8_e4m3

BS = 8          # batch (== number of cores)
F = 64          # win_len (frames)
OBJ = 36        # objects per frame
D = 2048        # feature dim
N = F * OBJ     # 2304 objects per sample
NCH = N // 128  # 18 object-row chunks
NW = NCH // 2   # 9 aggregation windows (chunk pairs)
KC = D // 128   # 16 contraction chunks
KP = KC // 2    # 8 contraction pair-chunks (DoubleRow)
DW = 512        # matmul moving width (one PSUM bank of fp32)
ND = D // DW    # 4 output-column groups
LN_EPS = 1e-5

XS = 16.0       # activation fp8 scale (obj, oe, ve)
WS = 64.0       # weight fp8 scale (W_o)
ES = 16.0       # exp-weight downscale (stored w/ES)
MAGIC1 = 0x5F3759E0  # rsqrt bit-trick magic + 1 (for the ~x + (magic+1) form)

_BUILD_CACHE = {}


def _f32(x):
    return np.ascontiguousarray(np.asarray(x), dtype=np.float32)


def _klc_layout(w):
    """[D, M] -> [128(kl), KC*M] with element (kl, kc, m) = w[kc*128+kl, m]."""
    d, m = w.shape
    assert d == D
    return w.reshape(KC, 128, m).transpose(1, 0, 2).reshape(128, KC * m)


def _build(num_devices=BS):
    """Build + compile the SPMD Bass program (trivial-fill fast path)."""
    if num_devices in _BUILD_CACHE:
        return _BUILD_CACHE[num_devices]

    import concourse.bacc as bacc
    import concourse.tile as tile
    from concourse import mybir

    f32 = mybir.dt.float32
    f16 = mybir.dt.float16
    f8 = mybir.dt.float8e4
    i32 = mybir.dt.int32
    AF = mybir.ActivationFunctionType
    OP = mybir.AluOpType
    DR = mybir.MatmulPerfMode.DoubleRow

    nc = bacc.Bacc("TRN2", target_bir_lowering=False, debug=False,
                   num_devices=num_devices)

    objT_d = nc.dram_tensor("objT", [NCH, 128, KC * 128], f8, kind="ExternalInput").ap()
    wo_d = nc.dram_tensor("Wo", [128, KC * D], f8, kind="ExternalInput").ap()
    wv_d = nc.dram_tensor("Wv", [128, KC * D], f16, kind="ExternalInput").ap()
    vt_d = nc.dram_tensor("vT", [128, KC * F], f16, kind="ExternalInput").ap()
    out_d = nc.dram_tensor("out", [F, D], f32, kind="ExternalOutput").ap()

    exp_scale = 1.0 / (XS * XS * float(np.sqrt(D)))
    exp_bias = -float(np.log(ES))

    with tile.TileContext(nc) as tc:
        with tc.tile_pool(name="persist", bufs=1) as persist, \
             tc.tile_pool(name="stats", bufs=2) as stats_pool:

            ve_nat = persist.tile([F, D], f32)          # LN'd visual embedding
            veT8 = persist.tile([128, KC, F], f8)       # fp8(16*ve) transposed
            oe8 = persist.tile([128, NCH, D], f8)       # fp8(16*oe) natural
            expT8 = persist.tile([128, NCH, F], f8)     # fp8(w/16), [n, f]
            ones8 = persist.tile([128, 2, 1], f8)
            nc.vector.memset(ones8, 1.0)
            ebias128 = persist.tile([128, 1], f32)
            nc.vector.memset(ebias128, exp_bias)

            def layer_norm_to(t_in, rows, out_tile, out_scale=1.0):
                """LN over free dim of t_in[:rows] -> out_tile * out_scale."""
                st = stats_pool.tile([128, ND, nc.vector.BN_STATS_DIM], f32,
                                     tag="st")
                for j in range(ND):
                    nc.vector.bn_stats(out=st[:rows, j, :],
                                       in_=t_in[:rows, j * DW:(j + 1) * DW])
                mvr = stats_pool.tile([128, 8], f32, tag="mvr")
                nc.vector.bn_aggr(out=mvr[:rows, 0:2], in_=st[:rows])
                # rstd = rsqrt(var + eps) on DVE: bit-trick + 2 Newton steps.
                mu = mvr[:rows, 0:1]
                var = mvr[:rows, 1:2]
                rstd = mvr[:rows, 2:3]
                vh = mvr[:rows, 3:4]
                r = mvr[:rows, 4:5]
                t = mvr[:rows, 5:6]
                v = mvr[:rows, 6:7]
                nc.vector.tensor_scalar(out=vh, in0=var, scalar1=0.5,
                                        scalar2=0.5 * LN_EPS,
                                        op0=OP.mult, op1=OP.add)
                nc.vector.tensor_scalar(out=v, in0=var, scalar1=LN_EPS,
                                        scalar2=None, op0=OP.add)
                nc.vector.tensor_scalar(out=r.bitcast(i32), in0=v.bitcast(i32),
                                        scalar1=1, scalar2=None,
                                        op0=OP.logical_shift_right)
                nc.vector.tensor_scalar(out=r.bitcast(i32), in0=r.bitcast(i32),
                                        scalar1=-1, scalar2=None,
                                        op0=OP.bitwise_xor)
                nc.vector.tensor_scalar(out=r.bitcast(i32), in0=r.bitcast(i32),
                                        scalar1=MAGIC1, scalar2=None,
                                        op0=OP.add)
                for last in (False, True):
                    nc.vector.tensor_scalar(out=t, in0=r, scalar1=r,
                                            scalar2=None, op0=OP.mult)
                    nc.vector.tensor_scalar(out=t, in0=t, scalar1=vh,
                                            scalar2=-1.0, op0=OP.mult,
                                            op1=OP.mult)
                    nc.vector.tensor_scalar(out=t, in0=t, scalar1=1.5,
                                            scalar2=None, op0=OP.add)
                    if last:
                        nc.vector.tensor_scalar(out=rstd, in0=r, scalar1=t,
                                                scalar2=float(out_scale),
                                                op0=OP.mult, op1=OP.mult)
                    else:
                        nc.vector.tensor_scalar(out=r, in0=r, scalar1=t,
                                                scalar2=None, op0=OP.mult)
                nc.vector.tensor_scalar(out=out_tile[:rows], in0=t_in[:rows],
                                        scalar1=mu, scalar2=rstd,
                                        op0=OP.subtract, op1=OP.mult)

            with tc.tile_pool(name="wo", bufs=1) as wop, \
                 tc.tile_pool(name="objs", bufs=2) as objp, \
                 tc.tile_pool(name="oe16", bufs=2) as oe16p, \
                 tc.tile_pool(name="wt16", bufs=2) as wt16p, \
                 tc.tile_pool(name="wt8", bufs=6) as wt8p, \
                 tc.tile_pool(name="tmpB", bufs=2) as tmpB, \
                 tc.tile_pool(name="ew", bufs=1) as ewp, \
                 tc.tile_pool(name="psB", bufs=2, space="PSUM") as psB:
                wo = wop.tile([128, KC, D], f8)

                # DMA plan: objT chunks + vT ride the scalar HWDGE queue
                # (2 chunks ahead); W_o then W_v stream on the sync queue;
                # all transposes ride the vector queue.
                obj_tiles = {}

                def load_objT(nch):
                    t = objp.tile([128, KC, 128], f8, name="objT", tag="objT")
                    nc.scalar.dma_start(out=t, in_=objT_d[nch])
                    obj_tiles[nch] = t

                load_objT(0)
                load_objT(1)
                for kc in range(KC):
                    nc.sync.dma_start(out=wo[:, kc, :],
                                      in_=wo_d[:, kc * D:(kc + 1) * D])

                wt8_tiles = {}

                def emit_chunk_B(nch):
                    objT_nc = obj_tiles.pop(nch)
                    if nch + 2 < NCH:
                        load_objT(nch + 2)
                    tB = tmpB.tile([128, D], f16, tag="tB")
                    for q in range(ND):
                        pq = psB.tile([128, DW], f32, tag="psb")
                        for p in range(KP):
                            nc.tensor.matmul(
                                pq,
                                lhsT=objT_nc[:, 2 * p:2 * p + 2, :],
                                rhs=wo[:, 2 * p:2 * p + 2,
                                       q * DW:(q + 1) * DW],
                                start=(p == 0), stop=(p == KP - 1),
                                perf_mode=DR)
                        nc.scalar.activation(out=tB[:, q * DW:(q + 1) * DW],
                                             in_=pq, func=AF.Tanh,
                                             scale=1.0 / (XS * WS))
                    oe16 = oe16p.tile([128, D], f16, tag="oe16")
                    layer_norm_to(tB, 128, oe16, out_scale=XS)
                    # fp8 natural copy for D's moving operand
                    nc.vector.tensor_copy(out=oe8[:, nch, :], in_=oe16)
                    # transposed fp16 -> fp8 for C's stationary operand
                    wt16 = wt16p.tile([128, KC, 128], f16, tag="wt16")
                    nc.scalar.dma_start(out=wt16, in_=oe16, transpose=True)
                    wt8 = wt8p.tile([128, KC, 128], f8, tag="wt8")
                    nc.vector.tensor_copy(out=wt8, in_=wt16)
                    wt8_tiles[nch] = wt8

                def emit_C(nch, psC):
                    wt = wt8_tiles.pop(nch)
                    slot = nch % 8
                    for p in range(KP):
                        nc.tensor.matmul(
                            psC[:, slot, :],
                            lhsT=wt[:, 2 * p:2 * p + 2, :],
                            rhs=veT8[:, 2 * p:2 * p + 2, :],
                            start=(p == 0), stop=(p == KP - 1),
                            perf_mode=DR)

                def emit_exp(psC, c0, n):
                    ew = ewp.tile([128, 8, F], f16, tag="ew")
                    nc.scalar.activation(out=ew[:, 0:n, :], in_=psC[:, 0:n, :],
                                         func=AF.Exp, scale=exp_scale,
                                         bias=ebias128)
                    nc.vector.tensor_copy(out=expT8[:, c0:c0 + n, :],
                                          in_=ew[:, 0:n, :])

                def emit_D(w, ps_agg, ps_sums):
                    lhs = expT8[:, 2 * w:2 * w + 2, :]
                    for dd in range(ND):
                        nc.tensor.matmul(
                            ps_agg[:, dd * DW:(dd + 1) * DW],
                            lhsT=lhs,
                            rhs=oe8[:, 2 * w:2 * w + 2, dd * DW:(dd + 1) * DW],
                            start=(w == 0), stop=(w == NW - 1),
                            perf_mode=DR)
                    nc.tensor.matmul(ps_sums, lhsT=lhs, rhs=ones8,
                                     start=(w == 0), stop=(w == NW - 1),
                                     perf_mode=DR)

                # ---- chunks 0-3 (W_o-arrival paced), then visual branch ----
                with tc.tile_pool(name="wv", bufs=8) as wvp, \
                     tc.tile_pool(name="vt", bufs=1) as vtp, \
                     tc.tile_pool(name="psA", bufs=1, space="PSUM") as psA, \
                     tc.tile_pool(name="tmpA", bufs=1) as tmpA:
                    vt = vtp.tile([128, KC, F], f16)
                    nc.scalar.dma_start(out=vt, in_=vt_d)

                    wv_slices = []
                    for kc in range(KC):
                        wv_k = wvp.tile([128, D], f16, tag="wvk")
                        nc.sync.dma_start(out=wv_k, in_=wv_d[:, kc * D:(kc + 1) * D])
                        wv_slices.append(wv_k)

                    for nch in range(4):
                        emit_chunk_B(nch)

                    # ---- phase A: visual branch (fp16) ------------------
                    ps_ve = psA.tile([F, D], f32)
                    for kc in range(KC):
                        for dd in range(ND):
                            nc.tensor.matmul(
                                ps_ve[:, dd * DW:(dd + 1) * DW],
                                lhsT=vt[:, kc, :],
                                rhs=wv_slices[kc][:, dd * DW:(dd + 1) * DW],
                                start=(kc == 0), stop=(kc == KC - 1))
                    tA = tmpA.tile([F, D], f32)
                    nc.scalar.activation(out=tA, in_=ps_ve, func=AF.Tanh)
                    layer_norm_to(tA, F, ve_nat)
                    ve16 = tmpB.tile([F, D], f16, tag="tB")
                    nc.vector.tensor_scalar(out=ve16, in0=ve_nat, scalar1=XS,
                                            scalar2=None, op0=OP.mult)
                    veT16 = wt16p.tile([128, KC, F], f16, tag="veT16")
                    nc.scalar.dma_start(out=veT16, in_=ve16, transpose=True)
                    nc.vector.tensor_copy(out=veT8, in_=veT16)

                # ---- chunks 4-17 with fused C / exp / D -----------------
                with tc.tile_pool(name="psC", bufs=1, space="PSUM") as psCp, \
                     tc.tile_pool(name="psD", bufs=1, space="PSUM") as psDp, \
                     tc.tile_pool(name="psS", bufs=1, space="PSUM") as psSp:
                    psC = psCp.tile([128, 8, F], f32)
                    ps_agg = psDp.tile([F, D], f32)
                    ps_sums = psSp.tile([F, 1], f32)

                    # C(k) at chunk k+1 for k >= 4; C(0..3) drain at 5..8;
                    # exp after chunks 8 and 16; D windows follow each batch.
                    sched_C = {n: [n - 1] for n in range(5, NCH)}
                    for k in range(4):
                        sched_C[k + 5].append(k)
                    sched_D = {9: [0], 10: [1], 11: [2], 12: [3],
                               16: [4], 17: [5, 6, 7]}

                    for nch in range(4, NCH):
                        emit_chunk_B(nch)
                        for k in sched_C.get(nch, []):
                            emit_C(k, psC)
                        if nch == 8:
                            emit_exp(psC, 0, 8)
                        if nch == 16:
                            emit_exp(psC, 8, 8)
                        for w in sched_D.get(nch, []):
                            emit_D(w, ps_agg, ps_sums)

                    # ---- tail: last chunk's C, exp, final window --------
                    emit_C(NCH - 1, psC)
                    emit_exp(psC, 16, 2)
                    emit_D(NW - 1, ps_agg, ps_sums)

                    # ---- finalize: rescale by 1/(16*sum), residual, LN --
                    fin = wt16p.tile([F, 2], f32, tag="fin")
                    nc.vector.tensor_scalar(out=fin[:, 0:1], in0=ps_sums,
                                            scalar1=ES, scalar2=None,
                                            op0=OP.mult)
                    nc.vector.reciprocal(out=fin[:, 1:2], in_=fin[:, 0:1])
                    tD = tmpB.tile([F, D], f32, tag="outf")
                    nc.vector.scalar_tensor_tensor(
                        out=tD, in0=ps_agg, scalar=fin[:, 1:2],
                        in1=ve_nat, op0=OP.mult, op1=OP.add)
                    nc.scalar.activation(out=tD, in_=tD, func=AF.Tanh)
                    out_f = tmpB.tile([F, D], f32, tag="outf")
                    layer_norm_to(tD, F, out_f)
                    nc.sync.dma_start(out=out_d, in_=out_f)

    nc.compile()
    _BUILD_CACHE[num_devices] = nc
    return nc


def _numpy_fallback(inputs):
    """Exact fp32 implementation for non-trivial bias/gain fills."""
    def ln(x, g, b, eps=LN_EPS):
        mu = x.mean(-1, keepdims=True)
        var = x.var(-1, keepdims=True)
        return (x - mu) / np.sqrt(var + eps) * g + b

    vf = _f32(inputs["visual_feats"])
    of = _f32(inputs["obj_feats"])
    W_v, b_v = _f32(inputs["W_v"]), _f32(inputs["b_v"])
    W_o, b_o = _f32(inputs["W_o"]), _f32(inputs["b_o"])
    out = np.zeros((BS, F, D), np.float32)
    for i in range(BS):
        ve = ln(np.tanh(vf[i] @ W_v + b_v), _f32(inputs["ln_v_g"]), _f32(inputs["ln_v_b"]))
        oe = ln(np.tanh(of[i].reshape(N, D) @ W_o + b_o),
                _f32(inputs["ln_o_g"]), _f32(inputs["ln_o_b"]))
        adj = oe @ ve.T / np.sqrt(D)
        adj = np.exp(adj - adj.max(0, keepdims=True))
        adj /= adj.sum(0, keepdims=True)
        out[i] = ln(np.tanh(adj.T @ oe + ve),
                    _f32(inputs["ln_ov_g"]), _f32(inputs["ln_ov_b"]))
    return out


def _prep_core_inputs(visual, obj_flat, shared):
    """Host-side per-sample layout prep. visual [64,2048] f32, obj_flat [2304,2048] f32."""
    m = {
        "objT": np.ascontiguousarray(
            obj_flat.reshape(NCH, 128, KC, 128).transpose(0, 3, 2, 1) * np.float32(XS)
        ).astype(F8).reshape(NCH, 128, KC * 128),
        "vT": np.ascontiguousarray(
            _klc_layout(np.ascontiguousarray(visual.T))).astype(F16),
    }
    m.update(shared)
    return m


def run_kernel(inputs, trace=False):
    """Returns (out [8, 64, 2048] fp32, exec_time_ns or None)."""
    from concourse import bass_utils

    vecs = {k: _f32(inputs[k]) for k in
            ["b_v", "b_o", "ln_v_b", "ln_o_b", "ln_ov_b"]}
    gains = {k: _f32(inputs[k]) for k in ["ln_v_g", "ln_o_g", "ln_ov_g"]}
    trivial = (all(np.all(v == 0) for v in vecs.values())
               and all(np.all(g == 1) for g in gains.values()))
    if not trivial:
        return _numpy_fallback(inputs), None

    visual = _f32(inputs["visual_feats"])            # [8, 64, 2048]
    obj = _f32(inputs["obj_feats"])                  # [8, 64, 36, 2048]
    W_v = _f32(inputs["W_v"])
    W_o = _f32(inputs["W_o"])

    nc = _build()

    shared = {
        "Wo": np.ascontiguousarray(_klc_layout(W_o * np.float32(WS))).astype(F8),
        "Wv": np.ascontiguousarray(_klc_layout(W_v)).astype(F16),
    }
    in_maps = [
        _prep_core_inputs(visual[c], obj[c].reshape(N, D), shared)
        for c in range(BS)
    ]

    res = bass_utils.run_bass_kernel_spmd(
        nc, in_maps, core_ids=list(range(BS)), trace=trace)
    out = np.stack([res.results[c]["out"] for c in range(BS)], axis=0)
    return out.astype(np.float32), res.exec_time_ns


def kernel(**inputs):
    out, _ = run_kernel(inputs, trace=False)
    return out
